# revision 1
# baseline (speedup 1.0000x reference)
"""MCTC relative-position self-attention on 8 Trainium2 NeuronCores.

Sharding: core = (batch b, head-pair hp): b = core//2, heads {2*hp, 2*hp+1}
of that batch. Each core computes full attention for its 2 heads.

Key trick: rel_pos_rotate(rel)[b,h,i,j] == rel[b,h, M-1+j-i, i], so with
D = q @ E^T of shape [S, L] (L = 2M-1), the rotated matrix is simply
D_flat viewed with row stride L-1 and offset M-1:
    rot[i, j] = D_flat[i*(L-1) + (M-1) + j]
which is a plain strided DMA from a DRAM scratch — no compute.

Matmuls run as float32r (full PE rate at N>=256). Softmax skips the
max-subtraction (scores are O(3), exp is safe in fp32); the 1/sqrt(hd)
scale is folded into the Exp activation's scale; row-sums come from the
activation's accum_out in the same instruction.
"""

import math
import sys

if "/opt/trn_rl_repo" not in sys.path:
    sys.path.insert(0, "/opt/trn_rl_repo")

import numpy as np

import concourse.bass as bass
import concourse.mybir as mybir
import concourse.tile as tile
from concourse import bacc
from concourse.bass_utils import run_bass_kernel_spmd
from concourse.masks import make_identity

S = 920
DMODEL = 1536
HD = 384
M = 920
L = 2 * M - 1  # 1839
NH_PER_CORE = 2

F32 = mybir.dt.float32
# float32r would be 4x faster on the PE but the BIR verifier requires
# producers to pre-round fp32r operands (bitcast alone is rejected).
MM_DT = mybir.dt.float32

P = 128
NS = 8  # ceil(920/128) s-chunks, last has 24 rows
ND = 12  # 1536/128 contraction chunks for projections
NF = 3  # 384/128 feature chunks
NQK = 460  # half of 920, one PSUM bank


def _pc(c):
    return min(P, S - c * P)


def _mm(nc, out, lhsT, rhs, **kw):
    nc.tensor.matmul(out, lhsT.bitcast(MM_DT), rhs.bitcast(MM_DT), **kw)


def build_kernel():
    nc = bacc.Bacc("TRN2", target_bir_lowering=False, debug=False)

    x_d = nc.dram_tensor("x", [S, DMODEL], F32, kind="ExternalInput")
    wq_d = nc.dram_tensor("wq", [DMODEL, NH_PER_CORE * HD], F32, kind="ExternalInput")
    wk_d = nc.dram_tensor("wk", [DMODEL, NH_PER_CORE * HD], F32, kind="ExternalInput")
    wv_d = nc.dram_tensor("wv", [DMODEL, NH_PER_CORE * HD], F32, kind="ExternalInput")
    et_d = nc.dram_tensor("et", [HD, L], F32, kind="ExternalInput")
    out_d = nc.dram_tensor("out", [NH_PER_CORE, S, HD], F32, kind="ExternalOutput")

    from contextlib import ExitStack

    with tile.TileContext(nc) as tc, ExitStack() as ctx:
            ep = ctx.enter_context
            xt_pool = ep(tc.tile_pool(name="xt", bufs=1))
            et_pool = ep(tc.tile_pool(name="et", bufs=1))
            xin_pool = ep(tc.tile_pool(name="xin", bufs=2))
            wch_pool = ep(tc.tile_pool(name="wchunk", bufs=6))
            wv_pool = ep(tc.tile_pool(name="wvres", bufs=1))
            qkt_pool = ep(tc.tile_pool(name="qkt", bufs=1))
            v_pool = ep(tc.tile_pool(name="vsb", bufs=1))
            dst_pool = ep(tc.tile_pool(name="dstage", bufs=3))
            sc_pool = ep(tc.tile_pool(name="sc", bufs=3))
            rel_pool = ep(tc.tile_pool(name="rel", bufs=2))
            pT_pool = ep(tc.tile_pool(name="pT", bufs=1))
            out_pool = ep(tc.tile_pool(name="outp", bufs=2))
            small_pool = ep(tc.tile_pool(name="small", bufs=1))
            pmm = ep(tc.tile_pool(name="pmm", bufs=4, space="PSUM"))
            pv = ep(tc.tile_pool(name="pv", bufs=2, space="PSUM"))
            pt = ep(tc.tile_pool(name="pt", bufs=2, space="PSUM"))
            dram_pool = ep(tc.tile_pool(name="dram", bufs=2, space="DRAM"))

            ident = small_pool.tile([P, P], F32, tag="ident")
            make_identity(nc, ident)

            # ---- load E^T [384, 1839] -> [128, 3, 1839] ----
            et_sb = et_pool.tile([P, NF, L], F32, tag="et")
            et_view = et_d.ap().rearrange("(j p) l -> p j l", p=P)
            for j in range(NF):
                half = L // 2
                nc.sync.dma_start(et_sb[:, j, :half], et_view[:, j, :half])
                nc.sync.dma_start(et_sb[:, j, half:], et_view[:, j, half:])

            # ---- X -> X^T via PE transposes: xt [128, 12, 920] ----
            xt_sb = xt_pool.tile([P, ND, S], F32, tag="xt")
            for c in range(NS):
                pc = _pc(c)
                x_in = xin_pool.tile([P, DMODEL], F32, tag="xin")
                nc.sync.dma_start(
                    x_in[:pc, : DMODEL // 2], x_d.ap()[c * P : c * P + pc, : DMODEL // 2]
                )
                nc.sync.dma_start(
                    x_in[:pc, DMODEL // 2 :], x_d.ap()[c * P : c * P + pc, DMODEL // 2 :]
                )
                for d in range(ND):
                    ps = pt.tile([P, P], F32, tag="pt")
                    nc.tensor.transpose(
                        ps[:P, :pc], x_in[:pc, d * P : (d + 1) * P], ident[:pc, :pc]
                    )
                    nc.vector.tensor_copy(xt_sb[:, d, c * P : c * P + pc], ps[:P, :pc])

            for h in range(NH_PER_CORE):
                hs = h * HD

                # ---- q^T / k^T projections: [384, 920] = W_chunk.T @ X^T ----
                qT_sb = qkt_pool.tile([P, NF, S], F32, tag="qT")
                kT_sb = qkt_pool.tile([P, NF, S], F32, tag="kT")
                for w_d, dst in ((wq_d, qT_sb), (wk_d, kT_sb)):
                    for m in range(NF):
                        ps0 = pmm.tile([P, NQK], F32, tag="pmm")
                        ps1 = pmm.tile([P, NQK], F32, tag="pmm")
                        for kd in range(ND):
                            wch = wch_pool.tile([P, P], F32, tag="wch")
                            nc.sync.dma_start(
                                wch[:],
                                w_d.ap()[
                                    kd * P : (kd + 1) * P, hs + m * P : hs + (m + 1) * P
                                ],
                            )
                            _mm(
                                nc, ps0[:], wch[:], xt_sb[:, kd, :NQK],
                                start=(kd == 0), stop=(kd == ND - 1),
                            )
                            _mm(
                                nc, ps1[:], wch[:], xt_sb[:, kd, NQK:],
                                start=(kd == 0), stop=(kd == ND - 1),
                            )
                        nc.vector.tensor_copy(dst[:, m, :NQK], ps0[:])
                        nc.vector.tensor_copy(dst[:, m, NQK:], ps1[:])

                # ---- v projection (natural layout): [920, 384] ----
                wv_sb = wv_pool.tile([P, ND, HD], F32, tag="wv")
                wv_view = wv_d.ap()[:, hs : hs + HD].rearrange("(j p) f -> p j f", p=P)
                nc.sync.dma_start(wv_sb[:, : ND // 2, :], wv_view[:, : ND // 2, :])
                nc.sync.dma_start(wv_sb[:, ND // 2 :, :], wv_view[:, ND // 2 :, :])
                v_sb = v_pool.tile([P, NS, HD], F32, tag="v")
                for c in range(NS):
                    pc = _pc(c)
                    ps = pv.tile([P, HD], F32, tag="pv")
                    for kd in range(ND):
                        _mm(
                            nc, ps[:pc, :], xt_sb[:, kd, c * P : c * P + pc],
                            wv_sb[:, kd, :],
                            start=(kd == 0), stop=(kd == ND - 1),
                        )
                    nc.vector.tensor_copy(v_sb[:pc, c, :], ps[:pc, :])

                # ---- D = q E^T into DRAM scratch (only needed l-columns) ----
                d_dram = dram_pool.tile([S, L], F32, tag="dscratch")
                d_flat = d_dram.rearrange("a b -> (a b)")
                for c in range(NS):
                    pc = _pc(c)
                    i_max = c * P + pc - 1
                    l_lo = (M - 1) - i_max
                    l_hi = (L - 1) - c * P + 1
                    width = l_hi - l_lo
                    nt = 3
                    base = width // nt
                    sizes = [base + (1 if i < width % nt else 0) for i in range(nt)]
                    off = l_lo
                    for w in sizes:
                        ps = pmm.tile([P, NQK], F32, tag="pmm")
                        for kd in range(NF):
                            _mm(
                                nc, ps[:pc, :w],
                                qT_sb[:, kd, c * P : c * P + pc],
                                et_sb[:, kd, off : off + w],
                                start=(kd == 0), stop=(kd == NF - 1),
                            )
                        dstg = dst_pool.tile([P, NQK], F32, tag="dstg")
                        nc.vector.tensor_copy(dstg[:pc, :w], ps[:pc, :w])
                        nc.sync.dma_start(
                            d_dram[c * P : c * P + pc, off : off + w], dstg[:pc, :w]
                        )
                        off += w

                # ---- scores + rel + exp (+row-sum) per q-chunk ----
                denom = small_pool.tile([P, NS], F32, tag=f"den{h}")
                rden = small_pool.tile([P, NS], F32, tag=f"rden{h}")
                sc_tiles = []
                for c in range(NS):
                    pc = _pc(c)
                    rel_sb = rel_pool.tile([P, S], F32, tag="rel")
                    skew = (
                        d_flat[
                            (M - 1) + c * P * (L - 1) :
                            (M - 1) + c * P * (L - 1) + pc * (L - 1)
                        ]
                        .rearrange("(p x) -> p x", x=L - 1)
                    )
                    nc.sync.dma_start(rel_sb[:pc, :NQK], skew[:, :NQK])
                    nc.sync.dma_start(rel_sb[:pc, NQK:S], skew[:, NQK:S])

                    sc_sb = sc_pool.tile([P, S], F32, tag="sc")
                    for n in range(2):
                        ps = pmm.tile([P, NQK], F32, tag="pmm")
                        for kd in range(NF):
                            _mm(
                                nc, ps[:pc, :],
                                qT_sb[:, kd, c * P : c * P + pc],
                                kT_sb[:, kd, n * NQK : (n + 1) * NQK],
                                start=(kd == 0), stop=(kd == NF - 1),
                            )
                        nc.vector.tensor_add(
                            sc_sb[:pc, n * NQK : (n + 1) * NQK],
                            ps[:pc, :],
                            rel_sb[:pc, n * NQK : (n + 1) * NQK],
                        )
                    nc.scalar.activation(
                        sc_sb[:pc, :],
                        sc_sb[:pc, :],
                        mybir.ActivationFunctionType.Exp,
                        scale=float(1.0 / math.sqrt(HD)),
                        accum_out=denom[:pc, c : c + 1],
                    )
                    nc.vector.reciprocal(rden[:pc, c : c + 1], denom[:pc, c : c + 1])
                    sc_tiles.append(sc_sb)

                # ---- transpose exp(scores) -> probsT [k-part, q] ----
                pT_sb = pT_pool.tile([P, NS, S], F32, tag="pT")
                for c in range(NS):
                    pc = _pc(c)
                    for kc in range(NS):
                        pkc = _pc(kc)
                        ps = pt.tile([P, P], F32, tag="pt")
                        nc.tensor.transpose(
                            ps[:pkc, :pc],
                            sc_tiles[c][:pc, kc * P : kc * P + pkc],
                            ident[:pc, :pc],
                        )
                        nc.vector.tensor_copy(
                            pT_sb[:pkc, kc, c * P : c * P + pc], ps[:pkc, :pc]
                        )

                # ---- ctx = probsT.T @ v, normalized by 1/rowsum ----
                for c in range(NS):
                    pc = _pc(c)
                    ps = pv.tile([P, HD], F32, tag="pv")
                    for kc in range(NS):
                        pkc = _pc(kc)
                        _mm(
                            nc, ps[:pc, :],
                            pT_sb[:pkc, kc, c * P : c * P + pc],
                            v_sb[:pkc, kc, :],
                            start=(kc == 0), stop=(kc == NS - 1),
                        )
                    o_sb = out_pool.tile([P, HD], F32, tag="o")
                    nc.vector.tensor_scalar_mul(
                        o_sb[:pc, :], ps[:pc, :], rden[:pc, c : c + 1]
                    )
                    nc.sync.dma_start(
                        out_d.ap()[h, c * P : c * P + pc, :], o_sb[:pc, :]
                    )

    nc.compile()
    return nc


_NC = None
LAST_RESULTS = None


def kernel(hidden_states, q_w, k_w, v_w, dist_emb):
    global _NC, LAST_RESULTS
    if _NC is None:
        _NC = build_kernel()

    hidden_states = np.asarray(hidden_states, dtype=np.float32)
    q_w = np.asarray(q_w, dtype=np.float32)
    k_w = np.asarray(k_w, dtype=np.float32)
    v_w = np.asarray(v_w, dtype=np.float32)
    dist_emb = np.asarray(dist_emb, dtype=np.float32)

    et = np.ascontiguousarray(dist_emb.T)
    in_maps = []
    for core in range(8):
        b, hp = core // 2, core % 2
        sl = slice(hp * NH_PER_CORE * HD, (hp + 1) * NH_PER_CORE * HD)
        in_maps.append(
            {
                "x": np.ascontiguousarray(hidden_states[b]),
                "wq": np.ascontiguousarray(q_w[:, sl]),
                "wk": np.ascontiguousarray(k_w[:, sl]),
                "wv": np.ascontiguousarray(v_w[:, sl]),
                "et": et,
            }
        )

    res = run_bass_kernel_spmd(_NC, in_maps, core_ids=list(range(8)))
    LAST_RESULTS = res

    B = hidden_states.shape[0]
    out = np.empty((B, S, 4 * HD), np.float32)
    for core in range(8):
        b, hp = core // 2, core % 2
        o = res.results[core]["out"]
        for j in range(NH_PER_CORE):
            h = hp * NH_PER_CORE + j
            out[b, :, h * HD : (h + 1) * HD] = o[j]
    return out



# revision 3
# speedup vs baseline: 2.8188x; 2.8188x over previous
"""MCTC relative-position self-attention on 8 Trainium2 NeuronCores.

Sharding: core = (batch b, head-pair hp): b = core//2, heads {2*hp, 2*hp+1}
of that batch. Each core computes full attention for its 2 heads.

v2: all GEMMs and PE transposes run in bfloat16 (1 cyc/row vs fp32's 4)
with fp32 PSUM accumulation; inputs are pre-rounded to bf16 on the host so
DMA traffic also halves. Softmax stays fp32 on the Act engine (exp input
fp32, output bf16 for the 1-cyc/row probs transpose; row-sums via accum_out).

Key trick: rel_pos_rotate(rel)[b,h,i,j] == rel[b,h, M-1+j-i, i], so with
D = q @ E^T of shape [S, L] (L = 2M-1), the rotated matrix is simply
D_flat viewed with row stride L-1 and offset M-1:
    rot[i, j] = D_flat[i*(L-1) + (M-1) + j]
which is a plain strided DMA from a DRAM scratch - no compute.
"""

import math
import sys

if "/opt/trn_rl_repo" not in sys.path:
    sys.path.insert(0, "/opt/trn_rl_repo")

import ml_dtypes
import numpy as np

import concourse.bass as bass
import concourse.mybir as mybir
import concourse.tile as tile
from concourse import bacc
from concourse.bass_utils import run_bass_kernel_spmd
from concourse.masks import make_identity

S = 920
DMODEL = 1536
HD = 384
M = 920
L = 2 * M - 1  # 1839
NH_PER_CORE = 2
NFEAT = NH_PER_CORE * HD  # 768

F32 = mybir.dt.float32
BF16 = mybir.dt.bfloat16

P = 128
NS = 8  # ceil(920/128) s-chunks, last has 24 rows
ND = 12  # 1536/128 contraction chunks for projections
NF = 3  # 384/128 feature chunks per head
NFH = 6  # 768/128 feature chunks for the head pair
NQK = 460  # half of 920, one PSUM bank


def _pc(c):
    return min(P, S - c * P)


def build_kernel():
    nc = bacc.Bacc("TRN2", target_bir_lowering=False, debug=False)

    x_d = nc.dram_tensor("x", [S, DMODEL], BF16, kind="ExternalInput")
    wq_d = nc.dram_tensor("wq", [DMODEL, NFEAT], BF16, kind="ExternalInput")
    wk_d = nc.dram_tensor("wk", [DMODEL, NFEAT], BF16, kind="ExternalInput")
    wv_d = nc.dram_tensor("wv", [DMODEL, NFEAT], BF16, kind="ExternalInput")
    et_d = nc.dram_tensor("et", [HD, L], BF16, kind="ExternalInput")
    out_d = nc.dram_tensor("out", [NH_PER_CORE, S, HD], F32, kind="ExternalOutput")

    from contextlib import ExitStack

    with tile.TileContext(nc) as tc, ExitStack() as ctx:
            ep = ctx.enter_context
            w_pool = ep(tc.tile_pool(name="w", bufs=1))
            xt_pool = ep(tc.tile_pool(name="xt", bufs=1))
            et_pool = ep(tc.tile_pool(name="et", bufs=1))
            xin_pool = ep(tc.tile_pool(name="xin", bufs=2))
            qkt_pool = ep(tc.tile_pool(name="qkt", bufs=1))
            v_pool = ep(tc.tile_pool(name="vsb", bufs=1))
            dst_pool = ep(tc.tile_pool(name="dstage", bufs=3))
            scf_pool = ep(tc.tile_pool(name="scf", bufs=2))
            scb_pool = ep(tc.tile_pool(name="scb", bufs=4))
            rel_pool = ep(tc.tile_pool(name="rel", bufs=2))
            pT_pool = ep(tc.tile_pool(name="pT", bufs=1))
            out_pool = ep(tc.tile_pool(name="outp", bufs=2))
            small_pool = ep(tc.tile_pool(name="small", bufs=1))
            pmm = ep(tc.tile_pool(name="pmm", bufs=4, space="PSUM"))
            pv = ep(tc.tile_pool(name="pv", bufs=2, space="PSUM"))
            pt = ep(tc.tile_pool(name="pt", bufs=2, space="PSUM"))
            dram_pool = ep(tc.tile_pool(name="dram", bufs=2, space="DRAM"))

            ident = small_pool.tile([P, P], BF16, tag="ident")
            make_identity(nc, ident)

            # ---- whole weight tensors -> SBUF (full 1536B rows per DMA) ----
            wq_sb = w_pool.tile([P, ND, NFEAT], BF16, tag="wq")
            wk_sb = w_pool.tile([P, ND, NFEAT], BF16, tag="wk")
            wv_sb = w_pool.tile([P, ND, NFEAT], BF16, tag="wv")
            for w_d, w_sb in ((wq_d, wq_sb), (wk_d, wk_sb), (wv_d, wv_sb)):
                w_view = w_d.ap().rearrange("(j p) f -> p j f", p=P)
                nc.sync.dma_start(w_sb[:, : ND // 2, :], w_view[:, : ND // 2, :])
                nc.sync.dma_start(w_sb[:, ND // 2 :, :], w_view[:, ND // 2 :, :])

            # ---- load E^T [384, 1839] -> [128, 3, 1839] ----
            et_sb = et_pool.tile([P, NF, L], BF16, tag="et")
            et_view = et_d.ap().rearrange("(j p) l -> p j l", p=P)
            for j in range(NF):
                half = L // 2
                nc.sync.dma_start(et_sb[:, j, :half], et_view[:, j, :half])
                nc.sync.dma_start(et_sb[:, j, half:], et_view[:, j, half:])

            # ---- X -> X^T via PE transposes: xt [128, 12, 920] ----
            xt_sb = xt_pool.tile([P, ND, S], BF16, tag="xt")
            for c in range(NS):
                pc = _pc(c)
                x_in = xin_pool.tile([P, DMODEL], BF16, tag="xin")
                nc.sync.dma_start(
                    x_in[:pc, : DMODEL // 2], x_d.ap()[c * P : c * P + pc, : DMODEL // 2]
                )
                nc.sync.dma_start(
                    x_in[:pc, DMODEL // 2 :], x_d.ap()[c * P : c * P + pc, DMODEL // 2 :]
                )
                for d in range(ND):
                    ps = pt.tile([P, P], BF16, tag="pt")
                    nc.tensor.transpose(
                        ps[:P, :pc], x_in[:pc, d * P : (d + 1) * P], ident[:pc, :pc]
                    )
                    nc.vector.tensor_copy(xt_sb[:, d, c * P : c * P + pc], ps[:P, :pc])

            # ---- q^T / k^T projections for BOTH heads: [768, 920] ----
            qT_sb = qkt_pool.tile([P, NFH, S], BF16, tag="qT")
            kT_sb = qkt_pool.tile([P, NFH, S], BF16, tag="kT")
            for w_sb, dst in ((wq_sb, qT_sb), (wk_sb, kT_sb)):
                for m in range(NFH):
                    ps0 = pmm.tile([P, NQK], F32, tag="pmm")
                    ps1 = pmm.tile([P, NQK], F32, tag="pmm")
                    for kd in range(ND):
                        wch = w_sb[:, kd, m * P : (m + 1) * P]
                        nc.tensor.matmul(
                            ps0[:], wch, xt_sb[:, kd, :NQK],
                            start=(kd == 0), stop=(kd == ND - 1),
                        )
                        nc.tensor.matmul(
                            ps1[:], wch, xt_sb[:, kd, NQK:],
                            start=(kd == 0), stop=(kd == ND - 1),
                        )
                    nc.vector.tensor_copy(dst[:, m, :NQK], ps0[:])
                    nc.vector.tensor_copy(dst[:, m, NQK:], ps1[:])

            # ---- v projection for BOTH heads (natural layout): [920, 768] ----
            v_sb = v_pool.tile([P, NS, NFEAT], BF16, tag="v")
            for c in range(NS):
                pc = _pc(c)
                for h2 in range(NH_PER_CORE):
                    ps = pv.tile([P, HD], F32, tag="pv")
                    for kd in range(ND):
                        nc.tensor.matmul(
                            ps[:pc, :], xt_sb[:, kd, c * P : c * P + pc],
                            wv_sb[:, kd, h2 * HD : (h2 + 1) * HD],
                            start=(kd == 0), stop=(kd == ND - 1),
                        )
                    nc.vector.tensor_copy(
                        v_sb[:pc, c, h2 * HD : (h2 + 1) * HD], ps[:pc, :]
                    )

            for h in range(NH_PER_CORE):
                hm = h * NF  # feature-chunk offset of this head in qT/kT

                # ---- D = q E^T into DRAM scratch (only needed l-columns) ----
                d_dram = dram_pool.tile([S, L], BF16, tag="dscratch")
                d_flat = d_dram.rearrange("a b -> (a b)")
                for c in range(NS):
                    pc = _pc(c)
                    i_max = c * P + pc - 1
                    l_lo = (M - 1) - i_max
                    l_hi = (L - 1) - c * P + 1
                    width = l_hi - l_lo
                    nt = 3
                    base = width // nt
                    sizes = [base + (1 if i < width % nt else 0) for i in range(nt)]
                    off = l_lo
                    for w in sizes:
                        ps = pmm.tile([P, NQK], F32, tag="pmm")
                        for kd in range(NF):
                            nc.tensor.matmul(
                                ps[:pc, :w],
                                qT_sb[:, hm + kd, c * P : c * P + pc],
                                et_sb[:, kd, off : off + w],
                                start=(kd == 0), stop=(kd == NF - 1),
                            )
                        dstg = dst_pool.tile([P, NQK], BF16, tag="dstg")
                        nc.scalar.copy(dstg[:pc, :w], ps[:pc, :w])
                        nc.sync.dma_start(
                            d_dram[c * P : c * P + pc, off : off + w], dstg[:pc, :w]
                        )
                        off += w

                # ---- scores + rel + exp (+row-sum) per q-chunk ----
                denom = small_pool.tile([P, NS], F32, tag=f"den{h}")
                rden = small_pool.tile([P, NS], F32, tag=f"rden{h}")
                sc_tiles = []
                for c in range(NS):
                    pc = _pc(c)
                    rel_sb = rel_pool.tile([P, S], BF16, tag="rel")
                    skew = (
                        d_flat[
                            (M - 1) + c * P * (L - 1) :
                            (M - 1) + c * P * (L - 1) + pc * (L - 1)
                        ]
                        .rearrange("(p x) -> p x", x=L - 1)
                    )
                    nc.sync.dma_start(rel_sb[:pc, :NQK], skew[:, :NQK])
                    nc.sync.dma_start(rel_sb[:pc, NQK:S], skew[:, NQK:S])

                    sc_f = scf_pool.tile([P, S], F32, tag="scf")
                    for n in range(2):
                        ps = pmm.tile([P, NQK], F32, tag="pmm")
                        for kd in range(NF):
                            nc.tensor.matmul(
                                ps[:pc, :],
                                qT_sb[:, hm + kd, c * P : c * P + pc],
                                kT_sb[:, hm + kd, n * NQK : (n + 1) * NQK],
                                start=(kd == 0), stop=(kd == NF - 1),
                            )
                        nc.vector.tensor_add(
                            sc_f[:pc, n * NQK : (n + 1) * NQK],
                            ps[:pc, :],
                            rel_sb[:pc, n * NQK : (n + 1) * NQK],
                        )
                    sc_b = scb_pool.tile([P, S], BF16, tag="scb")
                    nc.scalar.activation(
                        sc_b[:pc, :],
                        sc_f[:pc, :],
                        mybir.ActivationFunctionType.Exp,
                        scale=float(1.0 / math.sqrt(HD)),
                        accum_out=denom[:pc, c : c + 1],
                    )
                    nc.vector.reciprocal(rden[:pc, c : c + 1], denom[:pc, c : c + 1])
                    sc_tiles.append(sc_b)

                # ---- transpose exp(scores) -> probsT [k-part, q] ----
                pT_sb = pT_pool.tile([P, NS, S], BF16, tag="pT")
                for c in range(NS):
                    pc = _pc(c)
                    for kc in range(NS):
                        pkc = _pc(kc)
                        ps = pt.tile([P, P], BF16, tag="pt")
                        nc.tensor.transpose(
                            ps[:pkc, :pc],
                            sc_tiles[c][:pc, kc * P : kc * P + pkc],
                            ident[:pc, :pc],
                        )
                        nc.vector.tensor_copy(
                            pT_sb[:pkc, kc, c * P : c * P + pc], ps[:pkc, :pc]
                        )

                # ---- ctx = probsT.T @ v, normalized by 1/rowsum ----
                for c in range(NS):
                    pc = _pc(c)
                    ps = pv.tile([P, HD], F32, tag="pv")
                    for kc in range(NS):
                        pkc = _pc(kc)
                        nc.tensor.matmul(
                            ps[:pc, :],
                            pT_sb[:pkc, kc, c * P : c * P + pc],
                            v_sb[:pkc, kc, h * HD : (h + 1) * HD],
                            start=(kc == 0), stop=(kc == NS - 1),
                        )
                    o_sb = out_pool.tile([P, HD], F32, tag="o")
                    nc.vector.tensor_scalar_mul(
                        o_sb[:pc, :], ps[:pc, :], rden[:pc, c : c + 1]
                    )
                    nc.sync.dma_start(
                        out_d.ap()[h, c * P : c * P + pc, :], o_sb[:pc, :]
                    )

    nc.compile()
    return nc


_NC = None
LAST_RESULTS = None


def kernel(hidden_states, q_w, k_w, v_w, dist_emb):
    global _NC, LAST_RESULTS
    if _NC is None:
        _NC = build_kernel()

    bf16 = ml_dtypes.bfloat16
    hidden_states = np.asarray(hidden_states, dtype=np.float32)
    x_bf = hidden_states.astype(bf16)
    q_bf = np.asarray(q_w, dtype=np.float32).astype(bf16)
    k_bf = np.asarray(k_w, dtype=np.float32).astype(bf16)
    v_bf = np.asarray(v_w, dtype=np.float32).astype(bf16)
    et = np.ascontiguousarray(np.asarray(dist_emb, dtype=np.float32).T.astype(bf16))

    in_maps = []
    for core in range(8):
        b, hp = core // 2, core % 2
        sl = slice(hp * NFEAT, (hp + 1) * NFEAT)
        in_maps.append(
            {
                "x": np.ascontiguousarray(x_bf[b]),
                "wq": np.ascontiguousarray(q_bf[:, sl]),
                "wk": np.ascontiguousarray(k_bf[:, sl]),
                "wv": np.ascontiguousarray(v_bf[:, sl]),
                "et": et,
            }
        )

    res = run_bass_kernel_spmd(_NC, in_maps, core_ids=list(range(8)))
    LAST_RESULTS = res

    B = hidden_states.shape[0]
    out = np.empty((B, S, 4 * HD), np.float32)
    for core in range(8):
        b, hp = core // 2, core % 2
        o = res.results[core]["out"]
        for j in range(NH_PER_CORE):
            h = hp * NH_PER_CORE + j
            out[b, :, h * HD : (h + 1) * HD] = o[j]
    return out


# revision 5
# speedup vs baseline: 3.2982x; 1.1701x over previous
"""MCTC relative-position self-attention on 8 Trainium2 NeuronCores.

Sharding: core = (batch b, head-pair hp): b = core//2, heads {2*hp, 2*hp+1}
of that batch. Each core computes full attention for its 2 heads.

v3 (on top of v2's all-bf16 GEMM pipeline):
 - X is pre-transposed on the host, so X^T streams straight into SBUF
   (no PE transposes / DVE copies for it, shorter startup critical path).
 - DMA program order streams X^T then Wq/Wk per-k-chunk so the first
   projection matmuls start ~3us in; Wv/E^T arrive behind them.
 - PE order: qT, kT, D(h0), D(h1), v-proj, scores(h0), scores(h1),
   probsT+ctx(h0), probsT+ctx(h1) - the D DRAM round-trip (skew rotate
   trick) hides behind the v projection.
 - DMA issue load spread across engine queues: sync=loads, scalar=D
   writes, gpsimd=rel reads, vector=ctx outputs.
 - probsT transposes land pairwise in one PSUM tile -> half the copies.

Key trick: rel_pos_rotate(rel)[b,h,i,j] == rel[b,h, M-1+j-i, i], so with
D = q @ E^T of shape [S, L] (L = 2M-1), the rotated matrix is simply
D_flat viewed with row stride L-1 and offset M-1:
    rot[i, j] = D_flat[i*(L-1) + (M-1) + j]
which is a plain strided DMA from a DRAM scratch - no compute.
"""

import math
import sys

if "/opt/trn_rl_repo" not in sys.path:
    sys.path.insert(0, "/opt/trn_rl_repo")

import ml_dtypes
import numpy as np

import concourse.bass as bass
import concourse.mybir as mybir
import concourse.tile as tile
from concourse import bacc
from concourse.bass_utils import run_bass_kernel_spmd
from concourse.masks import make_identity

S = 920
DMODEL = 1536
HD = 384
M = 920
L = 2 * M - 1  # 1839
NH_PER_CORE = 2
NFEAT = NH_PER_CORE * HD  # 768

F32 = mybir.dt.float32
BF16 = mybir.dt.bfloat16

P = 128
NS = 8  # ceil(920/128) s-chunks, last has 24 rows
ND = 12  # 1536/128 contraction chunks for projections
NF = 3  # 384/128 feature chunks per head
NFH = 6  # 768/128 feature chunks for the head pair
NQK = 460  # half of 920, one PSUM bank


def _pc(c):
    return min(P, S - c * P)


def build_kernel():
    nc = bacc.Bacc("TRN2", target_bir_lowering=False, debug=False)

    xt_d = nc.dram_tensor("xt", [DMODEL, S], BF16, kind="ExternalInput")
    wq_d = nc.dram_tensor("wq", [DMODEL, NFEAT], BF16, kind="ExternalInput")
    wk_d = nc.dram_tensor("wk", [DMODEL, NFEAT], BF16, kind="ExternalInput")
    wv_d = nc.dram_tensor("wv", [DMODEL, NFEAT], BF16, kind="ExternalInput")
    et_d = nc.dram_tensor("et", [HD, L], BF16, kind="ExternalInput")
    out_d = nc.dram_tensor("out", [NH_PER_CORE, S, HD], F32, kind="ExternalOutput")

    from contextlib import ExitStack

    with tile.TileContext(nc) as tc, ExitStack() as ctx:
            ep = ctx.enter_context
            w_pool = ep(tc.tile_pool(name="w", bufs=1))
            xt_pool = ep(tc.tile_pool(name="xt", bufs=1))
            et_pool = ep(tc.tile_pool(name="et", bufs=1))
            qkt_pool = ep(tc.tile_pool(name="qkt", bufs=1))
            v_pool = ep(tc.tile_pool(name="vsb", bufs=1))
            dst_pool = ep(tc.tile_pool(name="dstage", bufs=3))
            scf_pool = ep(tc.tile_pool(name="scf", bufs=3))
            scb_pool = ep(tc.tile_pool(name="scb", bufs=8))
            rel_pool = ep(tc.tile_pool(name="rel", bufs=4))
            pT_pool = ep(tc.tile_pool(name="pT", bufs=1))
            out_pool = ep(tc.tile_pool(name="outp", bufs=2))
            small_pool = ep(tc.tile_pool(name="small", bufs=1))
            pmm = ep(tc.tile_pool(name="pmm", bufs=4, space="PSUM"))
            pv = ep(tc.tile_pool(name="pv", bufs=2, space="PSUM"))
            pt = ep(tc.tile_pool(name="pt", bufs=2, space="PSUM"))
            dram_pool = ep(tc.tile_pool(name="dram", bufs=2, space="DRAM"))

            ident = small_pool.tile([P, P], BF16, tag="ident")
            make_identity(nc, ident)

            # ---- X^T streams in first (host pre-transposed): [128,12,920] --
            xt_sb = xt_pool.tile([P, ND, S], BF16, tag="xt")
            xt_view = xt_d.ap().rearrange("(j p) s -> p j s", p=P)
            for kd in range(ND):
                nc.sync.dma_start(xt_sb[:, kd, :], xt_view[:, kd, :])

            # ---- weights stream per k-chunk: wq, wk now; wv later ----
            wq_sb = w_pool.tile([P, ND, NFEAT], BF16, tag="wq")
            wk_sb = w_pool.tile([P, ND, NFEAT], BF16, tag="wk")
            wv_sb = w_pool.tile([P, ND, NFEAT], BF16, tag="wv")
            wq_view = wq_d.ap().rearrange("(j p) f -> p j f", p=P)
            wk_view = wk_d.ap().rearrange("(j p) f -> p j f", p=P)
            wv_view = wv_d.ap().rearrange("(j p) f -> p j f", p=P)
            for kd in range(ND):
                nc.sync.dma_start(wq_sb[:, kd, :], wq_view[:, kd, :])
            for kd in range(ND):
                nc.sync.dma_start(wk_sb[:, kd, :], wk_view[:, kd, :])

            # ---- E^T [384, 1839] -> [128, 3, 1839] (needed at D phase) ----
            et_sb = et_pool.tile([P, NF, L], BF16, tag="et")
            et_view = et_d.ap().rearrange("(j p) l -> p j l", p=P)
            for j in range(NF):
                half = L // 2
                nc.sync.dma_start(et_sb[:, j, :half], et_view[:, j, :half])
                nc.sync.dma_start(et_sb[:, j, half:], et_view[:, j, half:])

            for kd in range(ND):
                nc.sync.dma_start(wv_sb[:, kd, :], wv_view[:, kd, :])

            # ---- q^T / k^T projections for BOTH heads: [768, 920] ----
            qT_sb = qkt_pool.tile([P, NFH, S], BF16, tag="qT")
            kT_sb = qkt_pool.tile([P, NFH, S], BF16, tag="kT")
            for w_sb, dst in ((wq_sb, qT_sb), (wk_sb, kT_sb)):
                for m in range(NFH):
                    ps0 = pmm.tile([P, NQK], F32, tag="pmm")
                    ps1 = pmm.tile([P, NQK], F32, tag="pmm")
                    for kd in range(ND):
                        wch = w_sb[:, kd, m * P : (m + 1) * P]
                        nc.tensor.matmul(
                            ps0[:], wch, xt_sb[:, kd, :NQK],
                            start=(kd == 0), stop=(kd == ND - 1),
                        )
                        nc.tensor.matmul(
                            ps1[:], wch, xt_sb[:, kd, NQK:],
                            start=(kd == 0), stop=(kd == ND - 1),
                        )
                    nc.vector.tensor_copy(dst[:, m, :NQK], ps0[:])
                    nc.vector.tensor_copy(dst[:, m, NQK:], ps1[:])

            # ---- D = q E^T into DRAM scratch for both heads ----
            d_drams = []
            for h in range(NH_PER_CORE):
                hm = h * NF
                d_dram = dram_pool.tile([S, L], BF16, tag="dscratch")
                d_drams.append(d_dram)
                for c in range(NS):
                    pc = _pc(c)
                    i_max = c * P + pc - 1
                    l_lo = (M - 1) - i_max
                    l_hi = (L - 1) - c * P + 1
                    width = l_hi - l_lo
                    nt = 3
                    base = width // nt
                    sizes = [base + (1 if i < width % nt else 0) for i in range(nt)]
                    off = l_lo
                    for w in sizes:
                        ps = pmm.tile([P, NQK], F32, tag="pmm")
                        for kd in range(NF):
                            nc.tensor.matmul(
                                ps[:pc, :w],
                                qT_sb[:, hm + kd, c * P : c * P + pc],
                                et_sb[:, kd, off : off + w],
                                start=(kd == 0), stop=(kd == NF - 1),
                            )
                        dstg = dst_pool.tile([P, NQK], BF16, tag="dstg")
                        nc.scalar.copy(dstg[:pc, :w], ps[:pc, :w])
                        nc.scalar.dma_start(
                            d_dram[c * P : c * P + pc, off : off + w], dstg[:pc, :w]
                        )
                        off += w

            # ---- v projection for BOTH heads (natural layout): [920, 768] --
            v_sb = v_pool.tile([P, NS, NFEAT], BF16, tag="v")
            for c in range(NS):
                pc = _pc(c)
                for h2 in range(NH_PER_CORE):
                    ps = pv.tile([P, HD], F32, tag="pv")
                    for kd in range(ND):
                        nc.tensor.matmul(
                            ps[:pc, :], xt_sb[:, kd, c * P : c * P + pc],
                            wv_sb[:, kd, h2 * HD : (h2 + 1) * HD],
                            start=(kd == 0), stop=(kd == ND - 1),
                        )
                    nc.vector.tensor_copy(
                        v_sb[:pc, c, h2 * HD : (h2 + 1) * HD], ps[:pc, :]
                    )

            # ---- scores + rel + exp (+row-sum) per head, per q-chunk ----
            denoms, rdens, sc_all = [], [], []
            for h in range(NH_PER_CORE):
                hm = h * NF
                d_flat = d_drams[h].rearrange("a b -> (a b)")
                denom = small_pool.tile([P, NS], F32, tag=f"den{h}")
                rden = small_pool.tile([P, NS], F32, tag=f"rden{h}")
                denoms.append(denom)
                rdens.append(rden)
                sc_tiles = []
                for c in range(NS):
                    pc = _pc(c)
                    rel_sb = rel_pool.tile([P, S], BF16, tag="rel")
                    skew = (
                        d_flat[
                            (M - 1) + c * P * (L - 1) :
                            (M - 1) + c * P * (L - 1) + pc * (L - 1)
                        ]
                        .rearrange("(p x) -> p x", x=L - 1)
                    )
                    nc.gpsimd.dma_start(rel_sb[:pc, :NQK], skew[:, :NQK])
                    nc.gpsimd.dma_start(rel_sb[:pc, NQK:S], skew[:, NQK:S])

                    sc_f = scf_pool.tile([P, S], F32, tag="scf")
                    for n in range(2):
                        ps = pmm.tile([P, NQK], F32, tag="pmm")
                        for kd in range(NF):
                            nc.tensor.matmul(
                                ps[:pc, :],
                                qT_sb[:, hm + kd, c * P : c * P + pc],
                                kT_sb[:, hm + kd, n * NQK : (n + 1) * NQK],
                                start=(kd == 0), stop=(kd == NF - 1),
                            )
                        nc.vector.tensor_add(
                            sc_f[:pc, n * NQK : (n + 1) * NQK],
                            ps[:pc, :],
                            rel_sb[:pc, n * NQK : (n + 1) * NQK],
                        )
                    sc_b = scb_pool.tile([P, S], BF16, tag="scb")
                    nc.scalar.activation(
                        sc_b[:pc, :],
                        sc_f[:pc, :],
                        mybir.ActivationFunctionType.Exp,
                        scale=float(1.0 / math.sqrt(HD)),
                        accum_out=denom[:pc, c : c + 1],
                    )
                    nc.vector.reciprocal(rden[:pc, c : c + 1], denom[:pc, c : c + 1])
                    sc_tiles.append(sc_b)
                sc_all.append(sc_tiles)

            # ---- probsT transposes (paired) + ctx per head ----
            for h in range(NH_PER_CORE):
                sc_tiles = sc_all[h]
                rden = rdens[h]
                pT_sb = pT_pool.tile([P, NS, S], BF16, tag="pT")
                for c0 in range(0, NS, 2):
                    pcs = [_pc(c0), _pc(c0 + 1)]
                    for kc in range(NS):
                        pkc = _pc(kc)
                        ps = pt.tile([P, 2, P], BF16, tag="pt")
                        for j, c in enumerate((c0, c0 + 1)):
                            pc = pcs[j]
                            nc.tensor.transpose(
                                ps[:pkc, j, :pc],
                                sc_tiles[c][:pc, kc * P : kc * P + pkc],
                                ident[:pc, :pc],
                            )
                        w2 = pcs[0] + pcs[1]
                        nc.vector.tensor_copy(
                            pT_sb[:pkc, kc, c0 * P : c0 * P + w2],
                            ps[:pkc, :, :].rearrange("p a b -> p (a b)")[:, :w2],
                        )

                for c in range(NS):
                    pc = _pc(c)
                    ps = pv.tile([P, HD], F32, tag="pv")
                    for kc in range(NS):
                        pkc = _pc(kc)
                        nc.tensor.matmul(
                            ps[:pc, :],
                            pT_sb[:pkc, kc, c * P : c * P + pc],
                            v_sb[:pkc, kc, h * HD : (h + 1) * HD],
                            start=(kc == 0), stop=(kc == NS - 1),
                        )
                    o_sb = out_pool.tile([P, HD], F32, tag="o")
                    nc.vector.tensor_scalar_mul(
                        o_sb[:pc, :], ps[:pc, :], rden[:pc, c : c + 1]
                    )
                    nc.gpsimd.dma_start(
                        out_d.ap()[h, c * P : c * P + pc, :], o_sb[:pc, :]
                    )

    nc.compile()
    return nc


_NC = None
LAST_RESULTS = None


def kernel(hidden_states, q_w, k_w, v_w, dist_emb):
    global _NC, LAST_RESULTS
    if _NC is None:
        _NC = build_kernel()

    bf16 = ml_dtypes.bfloat16
    hidden_states = np.asarray(hidden_states, dtype=np.float32)
    x_bf = hidden_states.astype(bf16)
    q_bf = np.asarray(q_w, dtype=np.float32).astype(bf16)
    k_bf = np.asarray(k_w, dtype=np.float32).astype(bf16)
    v_bf = np.asarray(v_w, dtype=np.float32).astype(bf16)
    et = np.ascontiguousarray(np.asarray(dist_emb, dtype=np.float32).T.astype(bf16))

    in_maps = []
    for core in range(8):
        b, hp = core // 2, core % 2
        sl = slice(hp * NFEAT, (hp + 1) * NFEAT)
        in_maps.append(
            {
                "xt": np.ascontiguousarray(x_bf[b].T),
                "wq": np.ascontiguousarray(q_bf[:, sl]),
                "wk": np.ascontiguousarray(k_bf[:, sl]),
                "wv": np.ascontiguousarray(v_bf[:, sl]),
                "et": et,
            }
        )

    res = run_bass_kernel_spmd(_NC, in_maps, core_ids=list(range(8)))
    LAST_RESULTS = res

    B = hidden_states.shape[0]
    out = np.empty((B, S, 4 * HD), np.float32)
    for core in range(8):
        b, hp = core // 2, core % 2
        o = res.results[core]["out"]
        for j in range(NH_PER_CORE):
            h = hp * NH_PER_CORE + j
            out[b, :, h * HD : (h + 1) * HD] = o[j]
    return out
